# revision 1
# baseline (speedup 1.0000x reference)
"""9x9 morphological dilation (sliding-window max, SAME padding) on Trainium2.

Input : label (16, 1024, 1024, 1) float32, values in [0, 1).
Output: same shape; out[b,i,j] = max over the 9x9 window centered at (i,j),
        clipped to the image (cv2-style border handling for dilate).

Strategy (per NeuronCore; batch is data-parallel over 8 cores, 2 images/core):
  - SBUF tile layout: 128 partitions x (16 rows x U cols).  Partition p holds
    img = p//64, row-block q = p%64 (image rows 16q..16q+15).  The free dim is
    r-major; each column chunk has U = cw+12 padded columns (+-4 halo, zero
    pads at image edges; zero is a valid -inf substitute since inputs >= 0).
    Chunk widths are [64, 224, 224, 224, 224, 64]: the narrow first chunk
    shortens the initial load the pipeline waits on, the narrow last chunk
    shortens the exposed final horizontal stage + stores.
  - Vertical 9-max: log tree (shifts +1,+2,+4,+1 rows) as free-dim-shifted
    tensor_max ops; the 16-row block boundaries are fed by small SBUF->SBUF
    partition-shifted DMA "carry" tiles (DVE cannot read shifted partitions).
  - Horizontal 9-max: van Herk / Gil-Werman with two masked
    tensor_tensor_scan ops (segmented running max, segment length 9; the mask
    multiplies the running state by 0 at block starts) + one merge tensor_max.
  - The vertical result R9[r] covers rows R..R+8, i.e. output row R+4; the
    recentering happens in the store DMA offsets.  Output rows 0..3 (clipped
    top windows) are built from tree intermediates and stashed into the
    otherwise-unused rows (p%64==63, r>=12) so they ride the same horizontal
    pass and stores.
  - Emission is software-pipelined: chunk i's horizontal stage is emitted
    after chunk i+1's vertical tree so the stash/carry DMA latencies hide
    under tree compute.  Loads are prefetched one chunk ahead on the ACT
    HWDGE ring; carries/stash/stores ride the SP ring.
"""

import numpy as np

B, H, W = 16, 1024, 1024
NCORES = 8
IMGS = 2            # images per core
RB = 16             # rows per partition
CHUNKS = [256, 256, 256, 256]   # output cols per chunk (sum = 1024)
assert sum(CHUNKS) == W
WIDTHS = sorted(set(CHUNKS))
UMAX = max(CHUNKS) + 12

_CACHE = {}


def _build(reps=1):
    import concourse.bacc as bacc
    import concourse.tile as tile
    import concourse.mybir as mybir

    f32 = mybir.dt.float32
    mx = mybir.AluOpType.max
    ml = mybir.AluOpType.mult

    nc = bacc.Bacc("TRN2", target_bir_lowering=False, debug=False, num_devices=1)
    x = nc.dram_tensor("x", [IMGS, H, W], f32, kind="ExternalInput").ap()
    y = nc.dram_tensor("y", [IMGS, H, W], f32, kind="ExternalOutput").ap()

    xv = [x[i].rearrange("(q r) c -> q r c", r=RB) for i in range(IMGS)]

    chunk_off = np.cumsum([0] + CHUNKS[:-1]).tolist()

    with tile.TileContext(nc) as tc:
        with (
            tc.tile_pool(name="px", bufs=2) as px,
            tc.tile_pool(name="pa", bufs=2) as pa,
            tc.tile_pool(name="pb", bufs=3) as pb,
            tc.tile_pool(name="pd", bufs=1) as pd,
            tc.tile_pool(name="ptop", bufs=1) as ptop,
            tc.tile_pool(name="pconst", bufs=1) as pconst,
        ):
            # --- persistent carry tiles (2 ping-pong sets, sized for UMAX) ---
            # rows 63 and 127 stay zero (image-bottom clamp)
            carr = []
            for s in range(2):
                cset = []
                for nm, k in (("xc1", 1), ("t2c", 2), ("t4c", 4), ("t8c", 1)):
                    t = pconst.tile([128, k * UMAX], f32, tag=f"{nm}_{s}")
                    nc.gpsimd.memset(t[:], 0.0)
                    cset.append(t.rearrange("p (j u) -> p j u", u=UMAX))
                carr.append(cset)

            # --- per-width masks for the segmented horizontal scans ---
            # Mp: 0.0 where u % U == 4 + 9k (prefix block starts)
            # Ms: 0.0 where u % U == 3 + 9k (suffix block ends)
            masks = {}
            for cw in WIDTHS:
                u = cw + 12
                mp = pconst.tile([128, RB * u], f32, tag=f"mp{cw}")
                ms = pconst.tile([128, RB * u], f32, tag=f"ms{cw}")
                mp3 = mp.rearrange("p (r u) -> p r u", u=u)
                ms3 = ms.rearrange("p (r u) -> p r u", u=u)
                nc.gpsimd.memset(mp[:], 1.0)
                nc.gpsimd.memset(ms[:], 1.0)
                nc.gpsimd.memset(mp3[:, :, 4:u:9], 0.0)
                nc.gpsimd.memset(ms3[:, :, 3:u:9], 0.0)
                masks[cw] = (mp, ms)

            xpend = {}

            def alloc_load(ch):
                cw = CHUNKS[ch]
                u = cw + 12
                c0 = chunk_off[ch]
                clo = max(0, c0 - 4)
                chi = min(W, c0 + cw + 8)
                ncols = chi - clo
                ulo = clo - (c0 - 4)
                X = px.tile([128, RB * u], f32, tag="x")
                x3 = X.rearrange("p (r u) -> p r u", u=u)
                if ulo > 0:
                    nc.vector.memset(x3[:, :, 0:ulo], 0.0)
                if ulo + ncols < u:
                    nc.vector.memset(x3[:, :, ulo + ncols:u], 0.0)
                row_groups = [(0, 4), (4, 8), (8, 12), (12, RB)] if ch == 0 else [(0, RB)]
                for rlo, rhi in row_groups:
                    for img in range(IMGS):
                        b = 64 * img
                        nc.scalar.dma_start(
                            out=x3[b:b + 64, rlo:rhi, ulo:ulo + ncols],
                            in_=xv[img][:, rlo:rhi, clo:chi],
                        )
                return x3

            def emit_tree(it):
                ch = it % len(CHUNKS)
                cw = CHUNKS[ch]
                u = cw + 12
                fs = RB * u
                x3 = xpend.pop(it)
                xc1_3, t2c_3, t4c_3, t8c_3 = carr[it % 2]

                def carry_copy(dst3, src3, nrows):
                    # dst[p] = src[p+1, 0:nrows] for p in 0..62 and 64..126
                    nc.sync.dma_start(out=dst3[0:63, 0:nrows, 0:u], in_=src3[1:64, 0:nrows, :])
                    nc.sync.dma_start(out=dst3[64:127, 0:nrows, 0:u], in_=src3[65:128, 0:nrows, :])

                T2 = pa.tile([128, fs], f32, tag="a")
                t2_3 = T2.rearrange("p (r u) -> p r u", u=u)
                if it == 0:
                    # first chunk: start on the first loaded row-quarter while
                    # the rest of the very first load is still in flight
                    nc.vector.tensor_max(t2_3[:, 0:3, :], x3[:, 0:3, :], x3[:, 1:4, :])
                    carry_copy(xc1_3, x3, 1)
                    nc.vector.tensor_max(t2_3[:, 3:7, :], x3[:, 3:7, :], x3[:, 4:8, :])
                    nc.vector.tensor_max(t2_3[:, 7:11, :], x3[:, 7:11, :], x3[:, 8:12, :])
                    nc.vector.tensor_max(t2_3[:, 11:15, :], x3[:, 11:15, :], x3[:, 12:16, :])
                else:
                    carry_copy(xc1_3, x3, 1)
                    nc.vector.tensor_max(t2_3[:, 0:15, :], x3[:, 0:15, :], x3[:, 1:16, :])
                nc.vector.tensor_max(t2_3[:, 15:16, :], x3[:, 15:16, :], xc1_3[:, 0:1, 0:u])

                T4 = pb.tile([128, fs], f32, tag="b")
                t4_3 = T4.rearrange("p (r u) -> p r u", u=u)
                carry_copy(t2c_3, t2_3, 2)
                nc.vector.tensor_max(t4_3[:, 0:14, :], t2_3[:, 0:14, :], t2_3[:, 2:16, :])
                nc.vector.tensor_max(t4_3[:, 14:16, :], t2_3[:, 14:16, :], t2c_3[:, 0:2, 0:u])

                # top rows 0..3 (vertical prefixes over rows 0..4+k) depend only
                # on X/T2/T4 -> computed early so the stash DMA hides in the tree
                TOP = ptop.tile([128, 4 * u], f32, tag="top")
                top3 = TOP.rearrange("p (r u) -> p r u", u=u)
                for img in range(IMGS):
                    b = 64 * img
                    nc.vector.tensor_max(top3[b:b + 1, 0:1, :], t4_3[b:b + 1, 0:1, :], x3[b:b + 1, 4:5, :])
                    nc.vector.tensor_max(top3[b:b + 1, 1:2, :], t4_3[b:b + 1, 0:1, :], t2_3[b:b + 1, 4:5, :])
                    nc.vector.tensor_max(top3[b:b + 1, 2:3, :], t4_3[b:b + 1, 0:1, :], t4_3[b:b + 1, 3:4, :])

                T8 = pa.tile([128, fs], f32, tag="a")
                t8_3 = T8.rearrange("p (r u) -> p r u", u=u)
                carry_copy(t4c_3, t4_3, 4)
                nc.vector.tensor_max(t8_3[:, 0:12, :], t4_3[:, 0:12, :], t4_3[:, 4:16, :])
                nc.vector.tensor_max(t8_3[:, 12:16, :], t4_3[:, 12:16, :], t4c_3[:, 0:4, 0:u])
                for img in range(IMGS):
                    b = 64 * img
                    nc.scalar.copy(top3[b:b + 1, 3:4, :], t8_3[b:b + 1, 0:1, :])

                R9 = pb.tile([128, fs], f32, tag="b")
                r9_3 = R9.rearrange("p (r u) -> p r u", u=u)
                carry_copy(t8c_3, t8_3, 1)
                nc.vector.tensor_max(r9_3[:, 0:15, :], t8_3[:, 0:15, :], t8_3[:, 1:16, :])
                nc.vector.tensor_max(r9_3[:, 15:16, :], t8_3[:, 15:16, :], t8c_3[:, 0:1, 0:u])
                for img in range(IMGS):
                    b = 64 * img
                    nc.sync.dma_start(out=r9_3[b + 63:b + 64, 12:16, :], in_=top3[b:b + 1, 0:4, :])
                return (R9, r9_3)

            def emit_hstage(it, R9, r9_3, last=False):
                ch = it % len(CHUNKS)
                cw = CHUNKS[ch]
                u = cw + 12
                fs = RB * u
                c0 = chunk_off[ch]
                MPw, MSw = masks[cw]
                PH = pa.tile([128, fs], f32, tag="a")
                SH = pd.tile([128, fs], f32, tag="d")
                hf = fs // 2
                if last:
                    # split scans by row-halves so the final stores overlap
                    # the second half's scans (shrinks the exposed tail)
                    halves = [(0, hf), (hf, fs)]
                else:
                    halves = [(0, fs)]
                for lo, hi in halves:
                    nc.vector.tensor_tensor_scan(
                        PH[:, lo:hi], MPw[:, lo:hi], R9[:, lo:hi], 0.0, op0=ml, op1=mx
                    )
                    nc.vector.tensor_tensor_scan(
                        PH[:, lo:hi][:, ::-1] if False else SH[:, lo:hi][:, ::-1],
                        MSw[:, lo:hi][:, ::-1], R9[:, lo:hi][:, ::-1], 0.0,
                        op0=ml, op1=mx,
                    )

                OUT = pb.tile([128, fs], f32, tag="b")
                o3 = OUT.rearrange("p (r u) -> p r u", u=u)
                ph3 = PH.rearrange("p (r u) -> p r u", u=u)
                sh3 = SH.rearrange("p (r u) -> p r u", u=u)
                # merge + store per 4-row group so stores start early; the
                # last chunk's stores ride the (now idle) ACT ring so the
                # final drain doesn't wait behind the SP queue
                st = nc.sync
                if not last:
                    nc.vector.tensor_max(
                        o3[:, :, 4:4 + cw], sh3[:, :, 0:cw], ph3[:, :, 8:8 + cw]
                    )
                for g in range(4):
                    r0g, r1g = 4 * g, 4 * g + 4
                    if last:
                        nc.vector.tensor_max(
                            o3[:, r0g:r1g, 4:4 + cw],
                            sh3[:, r0g:r1g, 0:cw],
                            ph3[:, r0g:r1g, 8:8 + cw],
                        )
                    for img in range(IMGS):
                        b = 64 * img
                        ymain = y[img][4:4 + 63 * RB, c0:c0 + cw].rearrange(
                            "(q r) c -> q r c", r=RB
                        )
                        st.dma_start(
                            out=ymain[:, r0g:r1g, :], in_=o3[b:b + 63, r0g:r1g, 4:4 + cw]
                        )
                        if g < 3:
                            # bottom rows 1012..1023 live at (p%64==63, r 0..11)
                            ytail = y[img][1012 + 4 * g:1016 + 4 * g, c0:c0 + cw]
                            st.dma_start(
                                out=ytail, in_=o3[b + 63:b + 64, r0g:r1g, 4:4 + cw]
                            )
                        else:
                            # top rows 0..3 live in the stash (p%64==63, r 12..15)
                            ytop = y[img][0:4, c0:c0 + cw]
                            st.dma_start(
                                out=ytop, in_=o3[b + 63:b + 64, 12:16, 4:4 + cw]
                            )

            niter = len(CHUNKS) * reps
            xpend[0] = alloc_load(0)
            pending = None
            for it in range(niter):
                if it + 1 < niter:
                    xpend[it + 1] = alloc_load((it + 1) % len(CHUNKS))
                state = emit_tree(it)
                if pending is not None:
                    emit_hstage(*pending)
                pending = (it, *state)
            emit_hstage(*pending, last=True)

    nc.compile()
    return nc


def kernel(label):
    lab = np.ascontiguousarray(
        np.asarray(label, dtype=np.float32).reshape(B, H, W)
    )
    if "nc" not in _CACHE:
        _CACHE["nc"] = _build()
    nc = _CACHE["nc"]

    from concourse.bass_utils import run_bass_kernel_spmd

    in_maps = [{"x": lab[IMGS * c:IMGS * (c + 1)]} for c in range(NCORES)]
    res = run_bass_kernel_spmd(nc, in_maps, core_ids=list(range(NCORES)))
    out = np.concatenate([res.results[c]["y"] for c in range(NCORES)], axis=0)
    return out.reshape(B, H, W, 1)



# revision 13
# speedup vs baseline: 2.6518x; 2.6518x over previous
"""9x9 morphological dilation (sliding-window max, SAME padding) on Trainium2.

Input : label (16, 1024, 1024, 1) float32, values in [0, 1).
Output: same shape; out[b,i,j] = max over the 9x9 window centered at (i,j),
        clipped to the image (cv2-style border handling for dilate).

Strategy (per NeuronCore; batch is data-parallel over 8 cores, 2 images/core):
  - All device compute and HBM traffic is bf16.  max() commutes with
    round-to-nearest, so out == rn_bf16(exact out): rel err <= 2^-9 ~ 0.2%,
    far inside the 2e-2 gate.  The host converts f32->bf16 on the way in and
    bf16->f32 on the way out; device DMA traffic is halved and every DVE
    tensor_tensor op runs in the 2x_1P perf mode (2 elem/cycle/lane), which
    fp32 tensor_tensor cannot use.
  - Layout: partition p holds img = p%2, row-band q = p//2 (image rows
    16q..16q+15); free dim is r-major with U = cw+12 padded columns per
    chunk (+-4 halo, zero pads at image edges; zero is a valid -inf
    substitute since inputs >= 0).  Interleaving the two images on even/odd
    partitions makes "next row-band" = "partition p+2" for BOTH images, so
    every cross-partition carry is ONE partition-shifted DMA (and the
    image-bottom partitions 126/127 keep permanently-zero carry rows).
  - Each tree tile carries extra rows at the end (X:+1, T2:+2, T4:+4,
    T8:+1) that hold the next band's leading rows, so every tree level is a
    single full-tile tensor_max (no separate boundary op).
  - Vertical 9-max: log tree (row shifts +1,+2,+4,+1 applied to X,T2,T4,T8).
    Row shifts keep the column offset 0, so all operands stay 4B-aligned ->
    2x mode.
  - Horizontal 9-max: log tree with column shifts +1,+2,+4,+8.  A +1 bf16
    shift is 2B-misaligned and would drop the whole op to 1x, so the odd
    shift is materialized once per chunk as a shifted copy on the (otherwise
    idle) Activation engine: Y[m] = R9[m+1].  The rest (H2=max(R9,Y),
    H4=max(H2,H2>>2), H8=max(H4,H4>>4), OUT=max(H8,R9>>8)) is all even ->
    2x on the DVE.
  - HWDGE descriptor generation is a single serialized device costing
    ~625ns per dma_start regardless of size, so DMAs are maximally merged:
    per chunk 2 loads, 4 carries, 1 top-row stash, 4 stores.  The tiny
    top-row tensor_maxes run on the GpSimd(Pool) engine, off the DVE.
  - R9[r] covers rows R..R+8 -> output row R+4; recentering happens in the
    store offsets.  Output rows 0..3 (clipped top windows) are built from
    tree intermediates and stashed into the otherwise-unused rows
    (p in {126,127}, r>=12) so they ride the same horizontal pass/stores.
  - Emission is software-pipelined: chunk i's horizontal stage is emitted
    after chunk i+1's vertical tree so carry/stash DMA latencies and the
    ACT-engine shifted copy hide under tree compute.
"""

import numpy as np

B, H, W = 16, 1024, 1024
NCORES = 8
IMGS = 2            # images per core
RB = 16             # rows per partition
CW = 256            # output cols per chunk
NCH = 4             # chunks
assert CW * NCH == W
U = CW + 12

_CACHE = {}


def _build(reps=1):
    import concourse.bacc as bacc
    import concourse.tile as tile
    import concourse.mybir as mybir

    bf16 = mybir.dt.bfloat16

    nc = bacc.Bacc("TRN2", target_bir_lowering=False, debug=False, num_devices=1)
    x = nc.dram_tensor("x", [IMGS, H, W], bf16, kind="ExternalInput").ap()
    y = nc.dram_tensor("y", [IMGS, H, W], bf16, kind="ExternalOutput").ap()

    xv = [x[i].rearrange("(q r) c -> q r c", r=RB) for i in range(IMGS)]

    with tile.TileContext(nc) as tc:
        with tc.tile_pool(name="pt", bufs=1) as pt:

            def t3(rows, tag):
                t = pt.tile([128, rows * U], bf16, tag=tag)
                return t.rearrange("p (r u) -> p r u", u=U)

            XT = [t3(17, "x0"), t3(17, "x1")]
            T2T = t3(18, "t2")
            T4T = t3(20, "t4")
            T8T = t3(17, "t8")
            R9T = [t3(16, "r90"), t3(16, "r91")]
            YT = [t3(16, "y0"), t3(16, "y1")]
            H2T = t3(16, "h2")
            H4T = t3(16, "h4")
            H8T = t3(16, "h8")
            OT = [t3(16, "o0"), t3(16, "o1")]
            TPT = [t3(4, "tp0"), t3(4, "tp1")]

            # carry rows start zero; partitions 0:126 are rewritten by the
            # per-chunk carry DMAs, the image-bottom partitions (126,127)
            # keep the zeros forever (partition-sliced memsets are not legal
            # BIR, so clear all 128 partitions)
            for t, r0, r1 in ((XT[0], 16, 17), (XT[1], 16, 17), (T2T, 16, 18),
                              (T4T, 16, 20), (T8T, 16, 17)):
                nc.gpsimd.memset(t[:, r0:r1, :], 0.0)

            def load(it):
                ch = it % NCH
                c0 = CW * ch
                clo = max(0, c0 - 4)
                chi = min(W, c0 + CW + 8)
                ncols = chi - clo
                ulo = clo - (c0 - 4)
                x3 = XT[it % 2]
                if ulo > 0:
                    nc.gpsimd.memset(x3[:, 0:RB, 0:ulo], 0.0)
                if ulo + ncols < U:
                    nc.gpsimd.memset(x3[:, 0:RB, ulo + ncols:U], 0.0)
                row_groups = [(0, 4), (4, 8), (8, 12), (12, RB)] if it == 0 else [(0, RB)]
                for rlo, rhi in row_groups:
                    for img in range(IMGS):
                        nc.scalar.dma_start(
                            out=x3[img:img + 127:2, rlo:rhi, ulo:ulo + ncols],
                            in_=xv[img][:, rlo:rhi, clo:chi],
                        )
                return x3

            def emit_tree(it, x3, last=False):
                s = it % 2
                sync = nc.sync

                # X carry: row 16 <- next band's row 0 (both images at once)
                if it == 0:
                    # first chunk: T2 in row-quarters so compute starts while
                    # the very first load is still streaming in
                    nc.vector.tensor_max(T2T[:, 0:3, :], x3[:, 0:3, :], x3[:, 1:4, :])
                    sync.dma_start(out=x3[0:126, 16:17, :], in_=x3[2:128, 0:1, :])
                    nc.vector.tensor_max(T2T[:, 3:7, :], x3[:, 3:7, :], x3[:, 4:8, :])
                    nc.vector.tensor_max(T2T[:, 7:11, :], x3[:, 7:11, :], x3[:, 8:12, :])
                    nc.vector.tensor_max(T2T[:, 11:16, :], x3[:, 11:16, :], x3[:, 12:17, :])
                else:
                    sync.dma_start(out=x3[0:126, 16:17, :], in_=x3[2:128, 0:1, :])
                    nc.vector.tensor_max(T2T[:, 0:16, :], x3[:, 0:16, :], x3[:, 1:17, :])

                sync.dma_start(out=T2T[0:126, 16:18, :], in_=T2T[2:128, 0:2, :])
                nc.vector.tensor_max(T4T[:, 0:16, :], T2T[:, 0:16, :], T2T[:, 2:18, :])

                # top output rows 0..2 for both images (partitions 0,1 = q0)
                tp = TPT[s]
                nc.vector.tensor_max(tp[0:2, 0:1, :], T4T[0:2, 0:1, :], x3[0:2, 4:5, :])
                nc.vector.tensor_max(tp[0:2, 1:2, :], T4T[0:2, 0:1, :], T2T[0:2, 4:5, :])
                nc.vector.tensor_max(tp[0:2, 2:3, :], T4T[0:2, 0:1, :], T4T[0:2, 3:4, :])

                sync.dma_start(out=T4T[0:126, 16:20, :], in_=T4T[2:128, 0:4, :])
                nc.vector.tensor_max(T8T[:, 0:16, :], T4T[:, 0:16, :], T4T[:, 4:20, :])

                nc.scalar.copy(tp[0:2, 3:4, :], T8T[0:2, 0:1, :])

                sync.dma_start(out=T8T[0:126, 16:17, :], in_=T8T[2:128, 0:1, :])
                r9 = R9T[s]
                if last:
                    # split R9 + the ACT shifted copy by row halves so the
                    # final horizontal stage starts as early as possible
                    nc.vector.tensor_max(r9[:, 0:8, :], T8T[:, 0:8, :], T8T[:, 1:9, :])
                    nc.scalar.copy(YT[s][:, 0:8, 0:CW + 6], r9[:, 0:8, 1:CW + 7])
                    nc.vector.tensor_max(r9[:, 8:16, :], T8T[:, 8:16, :], T8T[:, 9:17, :])
                    sync.dma_start(out=r9[126:128, 12:16, :], in_=tp[0:2, 0:4, :])
                    nc.scalar.copy(YT[s][:, 8:16, 0:CW + 6], r9[:, 8:16, 1:CW + 7])
                else:
                    nc.vector.tensor_max(r9[:, 0:16, :], T8T[:, 0:16, :], T8T[:, 1:17, :])
                    # odd horizontal shift on the ACT engine: Y[m] = R9[m+1].
                    # rows 0:12 don't overlap the stash, so they copy while the
                    # stash DMA is still in flight; only rows 12:16 wait on it.
                    nc.scalar.copy(YT[s][:, 0:12, 0:CW + 6], r9[:, 0:12, 1:CW + 7])
                    # stash top rows into the unused (p 126/127, r 12..15) slots
                    sync.dma_start(out=r9[126:128, 12:16, :], in_=tp[0:2, 0:4, :])
                    nc.scalar.copy(YT[s][:, 12:16, 0:CW + 6], r9[:, 12:16, 1:CW + 7])

            def emit_hstage(it, last=False):
                s = it % 2
                c0 = CW * (it % NCH)
                r9 = R9T[s]
                o3 = OT[s]
                ymains = [
                    y[img][4:4 + 63 * RB, c0:c0 + CW].rearrange("(q r) c -> q r c", r=RB)
                    for img in range(IMGS)
                ]

                def store_main(rlo, rhi, split=False):
                    for img in range(IMGS):
                        # on the drain path, route one store via SWDGE (Pool)
                        # so the two final stores don't serialize on HWDGE
                        eng = nc.gpsimd if (split and img == 1) else nc.sync
                        eng.dma_start(
                            out=ymains[img][:, rlo:rhi, :],
                            in_=o3[img:img + 125:2, rlo:rhi, 0:CW],
                        )

                def store_tail():
                    # bottom rows 1012..1023 at (p 126/127, r 0..11); SWDGE
                    # (Pool) path keeps these small stores off the HWDGE queue
                    nc.gpsimd.dma_start(
                        out=y[:, 1012:1024, c0:c0 + CW], in_=o3[126:128, 0:12, 0:CW]
                    )

                def store_top():
                    # top rows 0..3 from the stash (p 126/127, r 12..15)
                    nc.gpsimd.dma_start(
                        out=y[:, 0:4, c0:c0 + CW], in_=o3[126:128, 12:16, 0:CW]
                    )

                def htree(hlo, hhi):
                    nc.vector.tensor_max(
                        H2T[:, hlo:hhi, 0:CW + 6],
                        r9[:, hlo:hhi, 0:CW + 6],
                        YT[s][:, hlo:hhi, 0:CW + 6],
                    )
                    nc.vector.tensor_max(
                        H4T[:, hlo:hhi, 0:CW + 4],
                        H2T[:, hlo:hhi, 0:CW + 4],
                        H2T[:, hlo:hhi, 2:CW + 6],
                    )
                    nc.vector.tensor_max(
                        H8T[:, hlo:hhi, 0:CW],
                        H4T[:, hlo:hhi, 0:CW],
                        H4T[:, hlo:hhi, 4:CW + 4],
                    )

                def merge(hlo, hhi):
                    nc.vector.tensor_max(
                        o3[:, hlo:hhi, 0:CW],
                        H8T[:, hlo:hhi, 0:CW],
                        r9[:, hlo:hhi, 8:CW + 8],
                    )

                if not last:
                    htree(0, 16)
                    merge(0, 16)
                    store_main(0, 16)
                    store_tail()
                    store_top()
                else:
                    # final chunk: drain in halves/quarters so stores overlap
                    # the remaining merges instead of queueing after them
                    htree(0, 8)
                    merge(0, 8)
                    store_main(0, 8)
                    htree(8, 16)
                    merge(8, 12)
                    store_main(8, 12)
                    store_tail()
                    merge(12, 16)
                    store_main(12, 16)
                    store_top()

            niter = NCH * reps
            xp = {0: load(0)}
            for it in range(niter):
                if it + 1 < niter:
                    xp[it + 1] = load(it + 1)
                emit_tree(it, xp.pop(it), last=(it == niter - 1))
                if it > 0:
                    emit_hstage(it - 1)
            emit_hstage(niter - 1, last=True)

    nc.compile()
    return nc


def kernel(label):
    import ml_dtypes

    lab = np.ascontiguousarray(
        np.asarray(label, dtype=np.float32).reshape(B, H, W)
    ).astype(ml_dtypes.bfloat16)
    if "nc" not in _CACHE:
        _CACHE["nc"] = _build()
    nc = _CACHE["nc"]

    from concourse.bass_utils import run_bass_kernel_spmd

    in_maps = [{"x": lab[IMGS * c:IMGS * (c + 1)]} for c in range(NCORES)]
    res = run_bass_kernel_spmd(nc, in_maps, core_ids=list(range(NCORES)))
    out = np.concatenate(
        [np.asarray(res.results[c]["y"]).astype(np.float32) for c in range(NCORES)],
        axis=0,
    )
    return out.reshape(B, H, W, 1)


# revision 14
# speedup vs baseline: 4.1275x; 1.5565x over previous
"""9x9 morphological dilation (sliding-window max, SAME padding) on Trainium2.

Input : label (16, 1024, 1024, 1) float32, values in [0, 1).
Output: same shape; out[b,i,j] = max over the 9x9 window centered at (i,j),
        clipped to the image (cv2-style border handling for dilate).

Strategy (per NeuronCore; batch is data-parallel over 8 cores, 2 images/core):
  - All device compute and HBM traffic is bf16.  max() commutes with
    round-to-nearest, so out == rn_bf16(exact out): rel err <= 2^-9 ~ 0.2%,
    far inside the 2e-2 gate.  The host converts f32->bf16 on the way in and
    bf16->f32 on the way out; device DMA traffic is halved and every DVE
    tensor_tensor op runs in the 2x_1P perf mode (2 elem/cycle/lane), which
    fp32 tensor_tensor cannot use.
  - Layout: partition p holds img = p%2, row-band q = p//2 (image rows
    16q..16q+15); free dim is r-major with u = cw+12 padded columns per
    chunk (+-4 halo, zero pads at image edges; zero is a valid -inf
    substitute since inputs >= 0).  Interleaving the two images on even/odd
    partitions makes "next row-band" = "partition p+2" for BOTH images, so
    every cross-partition carry is ONE partition-shifted DMA (and the
    image-bottom partitions 126/127 keep permanently-zero carry rows).
  - Each tree tile carries extra rows at the end (X:+1, T2:+2, T4:+4,
    T8:+1) that hold the next band's leading rows, so every tree level is a
    single full-tile tensor_max (no separate boundary op).
  - Vertical 9-max: log tree (row shifts +1,+2,+4,+1 applied to X,T2,T4,T8).
    Row shifts keep the column offset 0, so all operands stay 4B-aligned ->
    2x mode.
  - Horizontal 9-max: log tree with column shifts +1,+2,+4,+8.  A +1 bf16
    shift is 2B-misaligned and would drop the whole op to 1x, so the odd
    shift is materialized once per chunk as a shifted copy on the (otherwise
    idle) Activation engine: Y[m] = R9[m+1].  The rest (H2=max(R9,Y),
    H4=max(H2,H2>>2), H8=max(H4,H4>>4), OUT=max(H8,R9>>8)) is all even ->
    2x on the DVE.
  - HWDGE descriptor generation is a single serialized device costing
    ~625ns per dma_start regardless of size, so DMAs are maximally merged:
    per chunk 2 loads, 4 carries, 1 top-row stash, 4 stores (2 of them on
    the Pool/SWDGE path, which bypasses HWDGE).
  - R9[r] covers rows R..R+8 -> output row R+4; recentering happens in the
    store offsets.  Output rows 0..3 (clipped top windows) are built from
    tree intermediates and stashed into the otherwise-unused rows
    (p in {126,127}, r>=12) so they ride the same horizontal pass/stores.
  - Emission is software-pipelined: chunk i's horizontal stage is emitted
    after chunk i+1's vertical tree so carry/stash DMA latencies and the
    ACT-engine shifted copy hide under tree compute.
"""

import numpy as np

B, H, W = 16, 1024, 1024
NCORES = 8
IMGS = 2            # images per core
RB = 16             # rows per partition
CHUNKS = [344, 344, 336]
assert sum(CHUNKS) == W
NCH = len(CHUNKS)
UM = max(CHUNKS) + 12

_CACHE = {}


def _build(reps=1):
    import concourse.bacc as bacc
    import concourse.tile as tile
    import concourse.mybir as mybir

    bf16 = mybir.dt.bfloat16

    nc = bacc.Bacc("TRN2", target_bir_lowering=False, debug=False, num_devices=1)
    x = nc.dram_tensor("x", [IMGS, H, W], bf16, kind="ExternalInput").ap()
    y = nc.dram_tensor("y", [IMGS, H, W], bf16, kind="ExternalOutput").ap()

    xv = [x[i].rearrange("(q r) c -> q r c", r=RB) for i in range(IMGS)]
    chunk_off = np.cumsum([0] + CHUNKS[:-1]).tolist()

    with tile.TileContext(nc) as tc:
        with tc.tile_pool(name="pt", bufs=1) as pt:

            def t3(rows, tag):
                t = pt.tile([128, rows * UM], bf16, tag=tag)
                return t.rearrange("p (r u) -> p r u", u=UM)

            XT = [t3(17, "x0"), t3(17, "x1")]
            T2T = t3(18, "t2")
            T4T = t3(20, "t4")
            T8T = t3(17, "t8")
            R9T = [t3(16, "r90"), t3(16, "r91")]
            YT = [t3(16, "y0"), t3(16, "y1")]
            H2T = t3(16, "h2")
            H4T = t3(16, "h4")
            H8T = t3(16, "h8")
            OT = [t3(16, "o0"), t3(16, "o1")]
            TPT = [t3(4, "tp0"), t3(4, "tp1")]

            # carry rows start zero; partitions 0:126 are rewritten by the
            # per-chunk carry DMAs, the image-bottom partitions (126,127)
            # keep the zeros forever (partition-sliced memsets are not legal
            # BIR, so clear all 128 partitions)
            for t, r0, r1 in ((XT[0], 16, 17), (XT[1], 16, 17), (T2T, 16, 18),
                              (T4T, 16, 20), (T8T, 16, 17)):
                nc.gpsimd.memset(t[:, r0:r1, :], 0.0)

            def load(it):
                ch = it % NCH
                cw = CHUNKS[ch]
                u = cw + 12
                c0 = chunk_off[ch]
                clo = max(0, c0 - 4)
                chi = min(W, c0 + cw + 8)
                ncols = chi - clo
                ulo = clo - (c0 - 4)
                x3 = XT[it % 2]
                if ulo > 0:
                    nc.gpsimd.memset(x3[:, 0:RB, 0:ulo], 0.0)
                if ulo + ncols < u:
                    nc.gpsimd.memset(x3[:, 0:RB, ulo + ncols:u], 0.0)
                row_groups = [(0, 4), (4, 8), (8, 12), (12, RB)] if it == 0 else [(0, RB)]
                for rlo, rhi in row_groups:
                    for img in range(IMGS):
                        nc.scalar.dma_start(
                            out=x3[img:img + 127:2, rlo:rhi, ulo:ulo + ncols],
                            in_=xv[img][:, rlo:rhi, clo:chi],
                        )
                return x3

            def emit_tree(it, x3, last=False):
                s = it % 2
                cw = CHUNKS[it % NCH]
                u = cw + 12
                sync = nc.sync

                # X carry: row 16 <- next band's row 0 (both images at once)
                if it == 0:
                    # first chunk: T2 in row-quarters so compute starts while
                    # the very first load is still streaming in
                    nc.vector.tensor_max(T2T[:, 0:3, 0:u], x3[:, 0:3, 0:u], x3[:, 1:4, 0:u])
                    sync.dma_start(out=x3[0:126, 16:17, 0:u], in_=x3[2:128, 0:1, 0:u])
                    nc.vector.tensor_max(T2T[:, 3:7, 0:u], x3[:, 3:7, 0:u], x3[:, 4:8, 0:u])
                    nc.vector.tensor_max(T2T[:, 7:11, 0:u], x3[:, 7:11, 0:u], x3[:, 8:12, 0:u])
                    nc.vector.tensor_max(T2T[:, 11:16, 0:u], x3[:, 11:16, 0:u], x3[:, 12:17, 0:u])
                else:
                    sync.dma_start(out=x3[0:126, 16:17, 0:u], in_=x3[2:128, 0:1, 0:u])
                    nc.vector.tensor_max(T2T[:, 0:16, 0:u], x3[:, 0:16, 0:u], x3[:, 1:17, 0:u])

                sync.dma_start(out=T2T[0:126, 16:18, 0:u], in_=T2T[2:128, 0:2, 0:u])
                nc.vector.tensor_max(T4T[:, 0:16, 0:u], T2T[:, 0:16, 0:u], T2T[:, 2:18, 0:u])

                # top output rows 0..2 for both images (partitions 0,1 = q0)
                tp = TPT[s]
                nc.vector.tensor_max(tp[0:2, 0:1, 0:u], T4T[0:2, 0:1, 0:u], x3[0:2, 4:5, 0:u])
                nc.vector.tensor_max(tp[0:2, 1:2, 0:u], T4T[0:2, 0:1, 0:u], T2T[0:2, 4:5, 0:u])
                nc.vector.tensor_max(tp[0:2, 2:3, 0:u], T4T[0:2, 0:1, 0:u], T4T[0:2, 3:4, 0:u])

                sync.dma_start(out=T4T[0:126, 16:20, 0:u], in_=T4T[2:128, 0:4, 0:u])
                nc.vector.tensor_max(T8T[:, 0:16, 0:u], T4T[:, 0:16, 0:u], T4T[:, 4:20, 0:u])

                nc.scalar.copy(tp[0:2, 3:4, 0:u], T8T[0:2, 0:1, 0:u])

                sync.dma_start(out=T8T[0:126, 16:17, 0:u], in_=T8T[2:128, 0:1, 0:u])
                r9 = R9T[s]
                if last:
                    # split R9 + the ACT shifted copy by row halves so the
                    # final horizontal stage starts as early as possible
                    nc.vector.tensor_max(r9[:, 0:8, 0:u], T8T[:, 0:8, 0:u], T8T[:, 1:9, 0:u])
                    nc.scalar.copy(YT[s][:, 0:8, 0:cw + 6], r9[:, 0:8, 1:cw + 7])
                    nc.vector.tensor_max(r9[:, 8:16, 0:u], T8T[:, 8:16, 0:u], T8T[:, 9:17, 0:u])
                    sync.dma_start(out=r9[126:128, 12:16, 0:u], in_=tp[0:2, 0:4, 0:u])
                    nc.scalar.copy(YT[s][:, 8:16, 0:cw + 6], r9[:, 8:16, 1:cw + 7])
                else:
                    nc.vector.tensor_max(r9[:, 0:16, 0:u], T8T[:, 0:16, 0:u], T8T[:, 1:17, 0:u])
                    # odd horizontal shift on the ACT engine: Y[m] = R9[m+1].
                    # rows 0:12 don't overlap the stash, so they copy while the
                    # stash DMA is still in flight; only rows 12:16 wait on it.
                    nc.scalar.copy(YT[s][:, 0:12, 0:cw + 6], r9[:, 0:12, 1:cw + 7])
                    # stash top rows into the unused (p 126/127, r 12..15) slots
                    sync.dma_start(out=r9[126:128, 12:16, 0:u], in_=tp[0:2, 0:4, 0:u])
                    nc.scalar.copy(YT[s][:, 12:16, 0:cw + 6], r9[:, 12:16, 1:cw + 7])

            def emit_hstage(it, last=False):
                s = it % 2
                ch = it % NCH
                cw = CHUNKS[ch]
                c0 = chunk_off[ch]
                r9 = R9T[s]
                o3 = OT[s]
                ymains = [
                    y[img][4:4 + 63 * RB, c0:c0 + cw].rearrange("(q r) c -> q r c", r=RB)
                    for img in range(IMGS)
                ]

                def store_main(rlo, rhi, split=False):
                    for img in range(IMGS):
                        # on the drain path, route one store via SWDGE (Pool)
                        # so the two final stores don't serialize on HWDGE
                        eng = nc.gpsimd if (split and img == 1) else nc.sync
                        eng.dma_start(
                            out=ymains[img][:, rlo:rhi, :],
                            in_=o3[img:img + 125:2, rlo:rhi, 0:cw],
                        )

                def store_tail():
                    # bottom rows 1012..1023 at (p 126/127, r 0..11); SWDGE
                    # (Pool) path keeps these small stores off the HWDGE queue
                    nc.gpsimd.dma_start(
                        out=y[:, 1012:1024, c0:c0 + cw], in_=o3[126:128, 0:12, 0:cw]
                    )

                def store_top():
                    # top rows 0..3 from the stash (p 126/127, r 12..15)
                    nc.gpsimd.dma_start(
                        out=y[:, 0:4, c0:c0 + cw], in_=o3[126:128, 12:16, 0:cw]
                    )

                def htree(hlo, hhi):
                    nc.vector.tensor_max(
                        H2T[:, hlo:hhi, 0:cw + 6],
                        r9[:, hlo:hhi, 0:cw + 6],
                        YT[s][:, hlo:hhi, 0:cw + 6],
                    )
                    nc.vector.tensor_max(
                        H4T[:, hlo:hhi, 0:cw + 4],
                        H2T[:, hlo:hhi, 0:cw + 4],
                        H2T[:, hlo:hhi, 2:cw + 6],
                    )
                    nc.vector.tensor_max(
                        H8T[:, hlo:hhi, 0:cw],
                        H4T[:, hlo:hhi, 0:cw],
                        H4T[:, hlo:hhi, 4:cw + 4],
                    )

                def merge(hlo, hhi):
                    nc.vector.tensor_max(
                        o3[:, hlo:hhi, 0:cw],
                        H8T[:, hlo:hhi, 0:cw],
                        r9[:, hlo:hhi, 8:cw + 8],
                    )

                if not last:
                    htree(0, 16)
                    merge(0, 16)
                    store_main(0, 16)
                    store_tail()
                    store_top()
                else:
                    # final chunk: drain in halves/quarters so stores overlap
                    # the remaining merges instead of queueing after them
                    htree(0, 8)
                    merge(0, 8)
                    store_main(0, 8)
                    htree(8, 16)
                    merge(8, 12)
                    store_main(8, 12)
                    store_tail()
                    merge(12, 16)
                    store_main(12, 16, split=True)
                    store_top()

            niter = NCH * reps
            xp = {0: load(0)}
            for it in range(niter):
                if it + 1 < niter:
                    xp[it + 1] = load(it + 1)
                emit_tree(it, xp.pop(it), last=(it == niter - 1))
                if it > 0:
                    emit_hstage(it - 1)
            emit_hstage(niter - 1, last=True)

    nc.compile()
    return nc


def kernel(label):
    import ml_dtypes

    lab = np.ascontiguousarray(
        np.asarray(label, dtype=np.float32).reshape(B, H, W)
    ).astype(ml_dtypes.bfloat16)
    if "nc" not in _CACHE:
        _CACHE["nc"] = _build()
    nc = _CACHE["nc"]

    from concourse.bass_utils import run_bass_kernel_spmd

    in_maps = [{"x": lab[IMGS * c:IMGS * (c + 1)]} for c in range(NCORES)]
    res = run_bass_kernel_spmd(nc, in_maps, core_ids=list(range(NCORES)))
    out = np.concatenate(
        [np.asarray(res.results[c]["y"]).astype(np.float32) for c in range(NCORES)],
        axis=0,
    )
    return out.reshape(B, H, W, 1)
